# revision 26
# baseline (speedup 1.0000x reference)
"""Multi-head attention (B=2, S=2048, D=1024, H=16, causal-mask capable)
on 8 Trainium2 NeuronCores.

Sharding: batch x head-group tensor parallel. Core c handles batch b=c//4
and head group g=c%4 (4 heads, d' slice of 256). Wq/Wk/Wv are split
column-wise per head group, Wo row-wise; per-core partial outputs are
summed on host (plus bo).

Device dataflow (per core), all matmul operands in bf16 (1 cyc/row on the
PE at any moving size; rounding error ~0.2% per operand, far inside the
2e-2 budget; all accumulation stays fp32 in PSUM):
  - host supplies x^T (=[D, S]) per batch so contraction dims land on
    SBUF partitions with no on-device transposes; bf16 halves DMA bytes
  - qT/kT [d', s] and v [s, d'] projections accumulate over D in PSUM;
    q/v biases added on DVE, the 8 replicated k-bias writes on the ACT
    engine (Identity + per-partition bias AP) which idles during proj
  - scores^T[j, i] = kT^T-slice @ qT-slice per 128-key chunk; ACT exp
    (no max-subtraction needed: |scores| <~ 8 for unit-variance data)
  - causal diagonal blocks are sub-tiled into 128-query strips (drops
    ~15% of the score/ctx matmul rows) and masked by one static
    lower-triangle bf16 multiply on DVE instead of gpsimd affine_select
  - ctx^T accumulates v-chunk^T @ expS with an appended ones column so
    row 64 of PSUM carries the softmax denominator; normalize: DVE copy
    out of PSUM + fast reciprocal (NOT from PSUM: the custom DVE op
    misreads PSUM on hw) + gpsimd partition_broadcast + DVE multiply
  - output projection ctx^T-chunks @ Wo-chunks; bf16 partial [S, D] to
    HBM, upcast + summed on host
The causal schedule interleaves projection/output-projection work units
between attention score/ctx quads so the in-order PE queue always has
work that does not depend on the latest exp; PSUM is budgeted to exactly
8 banks (attn pairs 2x2 + proj/outproj 2x1 + ctx 2x1). DMA triggers cost
~600-800ns of sequencer time each and are spread across the Sync/Scalar/
GpSimd sequencers; constants are DVE-memset-built, and a dummy
activation at t=0 pre-loads the ACT function table (exp/identity/copy
share one table) during the DMA head.
"""

import os
import sys

import numpy as np

try:
    import concourse.bass as bass  # noqa: F401
except ImportError:
    sys.path.insert(0, "/opt/trn_rl_repo")

import concourse.bass as bass
import concourse.tile as tile
from concourse import bacc, mybir
from concourse.bass_utils import run_bass_kernel_spmd

# Optional NTFF profiling hook (only used when BASS_TRACE=1): the agent
# image's antenv package lacks axon_hooks, so register an equivalent.
try:
    import antenv.axon_hooks  # noqa: F401
except ImportError:
    try:
        import types

        import trn_agent_boot.trn_boot as _tb

        _h = _tb._ntff_profile_via_ctypes("/opt/axon/libaxon_pjrt.so")
        _m = types.ModuleType("antenv.axon_hooks")
        _m.get_axon_ntff_profile_hook = lambda: _h
        _m.set_axon_ntff_profile_hook = lambda h: None
        sys.modules["antenv.axon_hooks"] = _m
    except Exception:
        pass

B, S, D, H = 2, 2048, 1024, 16
DH = 64                 # head dim
HLOC = 4                # heads per core
DLOC = HLOC * DH        # 256 d' per core
KC = 8                  # contraction chunks of 128 over D
ST = 512                # s-tile (matmul moving size)
NST = S // ST           # 4
JC = S // 128           # 16 key chunks
NCORES = 8

F32 = mybir.dt.float32
BF16 = mybir.dt.bfloat16
NPBF16 = mybir.dt.np(BF16)

LAST_EXEC_TIME_NS = None
_NC_CACHE = {}


def _to_bf16(x: np.ndarray) -> np.ndarray:
    """Round fp32 -> bf16 nearest-even, returned as ml_dtypes.bfloat16."""
    u = np.ascontiguousarray(x, np.float32).view(np.uint32)
    lsb = (u >> np.uint32(16)) & np.uint32(1)
    out = ((u + np.uint32(0x7FFF) + lsb) >> np.uint32(16)).astype(np.uint16)
    return out.view(NPBF16)


def _xT_layout(x2d: np.ndarray) -> np.ndarray:
    """[S, D] -> [128, NST, KC, ST] with X[p,t,kc,s] = x[t*ST+s, kc*128+p],
    bf16. Gives 8KB-contiguous per-partition DMA descriptors."""
    v = x2d.reshape(NST, ST, KC, 128).transpose(3, 0, 2, 1)
    return _to_bf16(np.ascontiguousarray(v))


def _w_layout(w: np.ndarray, out: bool = False) -> np.ndarray:
    """[D, DLOC] -> [128, KC, DLOC] (or [DLOC, D] -> [128, 2, D]) with
    partition = contraction index within its 128-chunk; bf16."""
    kc = 2 if out else KC
    v = w.reshape(kc, 128, w.shape[1]).transpose(1, 0, 2)
    return _to_bf16(np.ascontiguousarray(v))


def _build(variant: str):
    """variant: 'causal' | 'zeros' | 'general'"""
    nc = bacc.Bacc("TRN2", target_bir_lowering=False, debug=False)

    xqT = nc.declare_dram_parameter("xqT", [128, NST, KC, ST], BF16, isOutput=False)
    xkT = nc.declare_dram_parameter("xkT", [128, NST, KC, ST], BF16, isOutput=False)
    xvT = nc.declare_dram_parameter("xvT", [128, NST, KC, ST], BF16, isOutput=False)
    wq = nc.declare_dram_parameter("wq", [128, KC, DLOC], BF16, isOutput=False)
    wk = nc.declare_dram_parameter("wk", [128, KC, DLOC], BF16, isOutput=False)
    wv = nc.declare_dram_parameter("wv", [128, KC, DLOC], BF16, isOutput=False)
    wo = nc.declare_dram_parameter("wo", [128, 2, D], BF16, isOutput=False)
    bq2 = nc.declare_dram_parameter("bq2", [128, 2], F32, isOutput=False)
    bk2 = nc.declare_dram_parameter("bk2", [128, 2], F32, isOutput=False)
    bv1 = nc.declare_dram_parameter("bv1", [1, DLOC], F32, isOutput=False)
    if variant == "general":
        maskTn = nc.declare_dram_parameter("maskTn", [S, S], F32, isOutput=False)
    out_d = nc.declare_dram_parameter("out", [S, D], BF16, isOutput=True)

    Exp = mybir.ActivationFunctionType.Exp
    Ident = mybir.ActivationFunctionType.Identity

    with tile.TileContext(nc) as tc:
        with tc.tile_pool(name="wpool", bufs=1) as wpool, \
             tc.tile_pool(name="xpool", bufs=1) as xpool, \
             tc.tile_pool(name="epool", bufs=3) as epool, \
             tc.tile_pool(name="opool", bufs=2) as opool, \
             tc.tile_pool(name="spool", bufs=1) as spool, \
             tc.tile_pool(name="mpool", bufs=1) as mpool, \
             tc.tile_pool(name="app", bufs=2, space="PSUM") as app, \
             tc.tile_pool(name="ppp", bufs=2, space="PSUM") as ppp, \
             tc.tile_pool(name="ctxp", bufs=2, space="PSUM") as ctxpool:

            # ---- startup: memset-built constants, ACT table prewarm,
            # DMA wavefront spread over sequencers, PE warmup ----
            vext = wpool.tile([128, JC, HLOC, 65], BF16, tag="vext")
            nc.vector.memset(vext[:, :, :, 64], 1.0)
            warm_sb = wpool.tile([128, 32], BF16, tag="warm")
            nc.vector.memset(warm_sb[:], 1.0)
            # Static masks for the sub-tiled diagonal block-row. Strip o
            # (128 queries) of a 512-query i-tile attends chunks c<=o of the
            # diagonal block-row; block (o,c) is full for c<o, in-block
            # lower-triangle for c==o, zero for c>o. One affine iota
            # 128*(strip-c)+(s-p) >= 0 encodes all three cases. mA covers
            # strips 0-1 (c in 0..1), mB strips 2-3 (c in 0..3).
            mA = wpool.tile([128, 2, 256], BF16, tag="mA")
            nc.vector.memset(mA[:], 1.0)
            nc.gpsimd.affine_select(
                out=mA[:].rearrange("p o (c s) -> p o c s", s=128),
                in_=mA[:].rearrange("p o (c s) -> p o c s", s=128),
                pattern=[[128, 2], [-128, 2], [1, 128]],
                compare_op=mybir.AluOpType.is_ge, fill=0.0,
                base=0, channel_multiplier=-1)
            mB = wpool.tile([128, 2, 512], BF16, tag="mB")
            nc.vector.memset(mB[:], 1.0)
            nc.gpsimd.affine_select(
                out=mB[:].rearrange("p o (c s) -> p o c s", s=128),
                in_=mB[:].rearrange("p o (c s) -> p o c s", s=128),
                pattern=[[128, 2], [-128, 4], [1, 128]],
                compare_op=mybir.AluOpType.is_ge, fill=0.0,
                base=256, channel_multiplier=-1)
            actwarm = wpool.tile([1, 8], F32, tag="actwarm")
            nc.scalar.activation(actwarm[:], warm_sb[0:1, 0:8], Exp)

            wq_sb = wpool.tile([128, KC, DLOC], BF16, tag="wq")
            wk_sb = wpool.tile([128, KC, DLOC], BF16, tag="wk")
            wv_sb = wpool.tile([128, KC, DLOC], BF16, tag="wv")
            wo_sb = wpool.tile([128, 2, D], BF16, tag="wo")
            bq_sb = wpool.tile([128, 2], F32, tag="bq")
            bk_sb = wpool.tile([128, 2], F32, tag="bk")
            bv_sb = wpool.tile([1, DLOC], F32, tag="bv")
            bvb = wpool.tile([128, DLOC], F32, tag="bvb")
            xq_t0 = xpool.tile([128, KC, ST], BF16, tag="xq")
            xk_t0 = xpool.tile([128, KC, ST], BF16, tag="xk")
            xv_t0 = xpool.tile([128, KC, ST], BF16, tag="xv", bufs=2)
            # first q-proj matmul needs only wq chunk 0 + xq chunk 0
            for kc2 in range(KC // 2):
                nc.scalar.dma_start(
                    wq_sb[:, 2 * kc2:2 * kc2 + 2, :],
                    wq[:, 2 * kc2:2 * kc2 + 2, :])
            for kc in range(KC):
                nc.sync.dma_start(
                    xq_t0[:, kc:kc + 1, :], xqT[:, 0, kc:kc + 1, :])
            nc.scalar.dma_start(bq_sb[:], bq2[:])
            nc.scalar.dma_start(bk_sb[:], bk2[:])
            nc.scalar.dma_start(wk_sb[:], wk[:])
            for half in range(2):
                nc.gpsimd.dma_start(
                    xk_t0[:, 4 * half:4 * half + 4, :],
                    xkT[:, 0, 4 * half:4 * half + 4, :])
            nc.scalar.dma_start(bv_sb[:], bv1[:])
            nc.scalar.dma_start(wv_sb[:], wv[:])
            for half in range(2):
                nc.gpsimd.dma_start(
                    xv_t0[:, 4 * half:4 * half + 4, :],
                    xvT[:, 0, 4 * half:4 * half + 4, :])
            nc.gpsimd.partition_broadcast(bvb[:], bv_sb[:])

            warm_ps = ppp.tile([128, ST], F32, tag="pj")
            # enough to bridge engine-boot (~7us) to first-chunk arrival
            # (~6us) and ramp the PE p-state; more would block the in-order
            # PE queue past data arrival
            for i in range(40):
                nc.tensor.matmul(
                    warm_ps[0:32, 0:32], warm_sb[:], warm_sb[:],
                    start=True, stop=True, skip_group_check=True)

            # persistent activation tensors
            qT = wpool.tile([128, 2, S], BF16, tag="qT")
            # kT stored replicated per head ([0:64] == [64:128]) so scores
            # matmuls run at K=128 like everything else (K-switches ~430ns)
            kT = wpool.tile([128, HLOC, S], BF16, tag="kT")
            ctxT = wpool.tile([128, 2, S], BF16, tag="ctxT")

            def emit_x_loads(t):
                # v first: with bufs=2 it is ungated and streams while the
                # sync-seq waits out the q/k buffer-reuse gates (bufs=1)
                xv_t = xpool.tile([128, KC, ST], BF16, tag="xv", bufs=2)
                nc.sync.dma_start(xv_t[:], xvT[:, t])
                xq_t = xpool.tile([128, KC, ST], BF16, tag="xq")
                xk_t = xpool.tile([128, KC, ST], BF16, tag="xk")
                nc.sync.dma_start(xq_t[:], xqT[:, t])
                nc.sync.dma_start(xk_t[:], xkT[:, t])
                return xq_t, xk_t, xv_t

            def proj_units(t, xs):
                """8 PE work units of ~0.9-1.8us each for tile t."""
                s0 = ST * t
                xq_t, xk_t, xv_t = xs

                def qk_unit(dst, dc, w_sb, b_sb, x_t, k_on_act=True):
                    def emit():
                        ps = ppp.tile([128, ST], F32, tag="pj")
                        for kc in range(KC):
                            nc.tensor.matmul(
                                ps[:], w_sb[:, kc, 128 * dc:128 * dc + 128],
                                x_t[:, kc, :],
                                start=(kc == 0), stop=(kc == KC - 1))
                        if dst is qT:
                            nc.vector.tensor_scalar_add(
                                out=dst[:, dc, s0:s0 + ST], in0=ps[:],
                                scalar1=b_sb[:, dc:dc + 1])
                        else:
                            # replicated k-bias writes (gpsimd cannot read
                            # PSUM): ACT when its phase is exp-light,
                            # else DVE
                            for half in range(2):
                                hsl = slice(64 * half, 64 * half + 64)
                                for rep in range(2):
                                    dsl = dst[64 * rep:64 * rep + 64,
                                              2 * dc + half, s0:s0 + ST]
                                    if k_on_act:
                                        nc.scalar.activation(
                                            dsl, ps[hsl, :], Ident,
                                            bias=b_sb[hsl, dc:dc + 1])
                                    else:
                                        nc.vector.tensor_scalar_add(
                                            out=dsl, in0=ps[hsl, :],
                                            scalar1=b_sb[hsl, dc:dc + 1])
                    return emit

                def v_unit(sc):
                    def emit():
                        ps = ppp.tile([128, ST], F32, tag="pj")
                        for kc in range(KC):
                            nc.tensor.matmul(
                                ps[:, 0:DLOC],
                                xv_t[:, kc, 128 * sc:128 * sc + 128],
                                wv_sb[:, kc, :],
                                start=(kc == 0), stop=(kc == KC - 1))
                        jc = 4 * t + sc
                        nc.vector.tensor_tensor(
                            out=vext[:, jc, :, 0:64],
                            in0=ps[:, 0:DLOC].rearrange(
                                "p (h d) -> p h d", d=DH),
                            in1=bvb[:].rearrange("p (h d) -> p h d", d=DH),
                            op=mybir.AluOpType.add)
                    return emit

                units = []
                units.append(qk_unit(qT, 0, wq_sb, bq_sb, xq_t))
                units.append(qk_unit(kT, 0, wk_sb, bk_sb, xk_t))
                units.append(qk_unit(qT, 1, wq_sb, bq_sb, xq_t))
                units.append(qk_unit(kT, 1, wk_sb, bk_sb, xk_t))
                for sc in range(4):
                    units.append(v_unit(sc))
                return units

            def outproj_units(it, last=False, act_drain=True):
                """4 sc groups x 2 et units of 2 matmuls + drain + store."""
                i0 = ST * it
                units = []
                obs = [None] * 4

                def op_unit(sc, et):
                    def emit():
                        if obs[sc] is None:
                            obs[sc] = opool.tile([128, D], BF16, tag="ob",
                                                 name="ob")
                        ob = obs[sc]
                        ps = ppp.tile([128, ST], F32, tag="pj")
                        for dc in range(2):
                            nc.tensor.matmul(
                                ps[:],
                                ctxT[:, dc, i0 + 128 * sc:i0 + 128 * sc + 128],
                                wo_sb[:, dc, ST * et:ST * et + ST],
                                start=(dc == 0), stop=(dc == 1))
                        if et == 0 and act_drain:
                            nc.scalar.copy(ob[:, ST * et:ST * et + ST], ps[:])
                        else:
                            nc.vector.tensor_copy(
                                out=ob[:, ST * et:ST * et + ST], in_=ps[:])
                        if et == 1:
                            nc.sync.dma_start(
                                out_d[i0 + 128 * sc:i0 + 128 * sc + 128, :],
                                ob[:])
                    return emit

                for sc in range(4):
                    for et in range(2):
                        units.append(op_unit(sc, et))
                return units

            def attn_steps_causal(p):
                """Generator of (emit_fn, injectable) steps for i-tile p.
                injectable=True marks points where foreign PE work may be
                inserted without hurting the score->exp->ctx pipeline."""
                i0 = ST * p
                for h in range(HLOC):
                    pb = 64 * (h % 2)
                    hc = h // 2
                    # tiles are allocated lazily at emission time (pool
                    # buffer-rotation deps only see already-emitted users)
                    tl = {}

                    def prep(h=h, pb=pb, hc=hc, tl=tl, i0=i0, p=p):
                        tl["cp"] = ctxpool.tile([65, ST], F32, tag="ctx",
                                                name="cp")
                        tl["qrep"] = epool.tile([128, ST], BF16, tag="qrep",
                                                bufs=3, name="qrep")
                        tl["esA"] = epool.tile([128, 2, 256], BF16, tag="esA",
                                               bufs=2, name="esA")
                        tl["esB"] = epool.tile([128, 2, 512], BF16, tag="esB",
                                               bufs=2, name="esB")
                        qrep, esA, esB = tl["qrep"], tl["esA"], tl["esB"]
                        nc.vector.tensor_copy(
                            out=qrep[0:64, :], in_=qT[pb:pb + 64, hc, i0:i0 + ST])
                        nc.vector.tensor_copy(
                            out=qrep[64:128, :], in_=qT[pb:pb + 64, hc, i0:i0 + ST])
                        # diagonal block-row, sub-tiled into 128-query
                        # strips: strip o needs chunks c<=o; computed as
                        # strips 0-1 x chunks 0-1 (sdA) and strips 2-3 x
                        # chunks 0-3 (sdB), masked by the static mA/mB
                        sdA = ppp.tile([128, ST], F32, tag="pj", name="sdA")
                        for o in range(2):
                            for c in range(2):
                                jc = 4 * p + c
                                nc.tensor.matmul(
                                    sdA[:, 256 * o + 128 * c:
                                        256 * o + 128 * c + 128],
                                    kT[:, h, 128 * jc:128 * jc + 128],
                                    qrep[:, 128 * o:128 * o + 128],
                                    start=True, stop=True,
                                    skip_group_check=True)
                        sdB = app.tile([128, 2, ST], F32, tag="mm")
                        for o in range(2):
                            for c in range(4):
                                jc = 4 * p + c
                                nc.tensor.matmul(
                                    sdB[:, o, 128 * c:128 * c + 128],
                                    kT[:, h, 128 * jc:128 * jc + 128],
                                    qrep[:, 256 + 128 * o:256 + 128 * o + 128],
                                    start=True, stop=True,
                                    skip_group_check=True)
                        nc.scalar.activation(
                            esA[:], sdA[:].rearrange("p (o x) -> p o x", o=2),
                            Exp)
                        nc.vector.tensor_tensor(
                            out=esA[:], in0=esA[:], in1=mA[:],
                            op=mybir.AluOpType.mult)
                        nc.scalar.activation(esB[:], sdB[:], Exp)
                        nc.vector.tensor_tensor(
                            out=esB[:], in0=esB[:], in1=mB[:],
                            op=mybir.AluOpType.mult)
                    yield prep, True

                    def s_quad(qd, h=h, tl=tl):
                        tiles = []
                        for half in range(2):
                            jc0 = 4 * qd + 2 * half
                            sp = app.tile([128, 2, ST], F32, tag="mm",
                                          name="sp")
                            for c in range(2):
                                jc = jc0 + c
                                nc.tensor.matmul(
                                    sp[:, c, :],
                                    kT[:, h, 128 * jc:128 * jc + 128],
                                    tl["qrep"][:],
                                    start=True, stop=True,
                                    skip_group_check=True)
                            es = epool.tile([128, 2, ST], BF16, tag="es",
                                            name="es")
                            nc.scalar.activation(es[:], sp[:], Exp)
                            tiles.append(es)
                        return tiles

                    def c_quad(qd, tiles, tl=tl, h=h):
                        for half, es in enumerate(tiles):
                            for c in range(2):
                                jc = 4 * qd + 2 * half + c
                                nc.tensor.matmul(
                                    tl["cp"][:], vext[:, jc, h, 0:65],
                                    es[:, c, :],
                                    start=(jc == 0), stop=False,
                                    skip_group_check=True)

                    es_prev = [None]

                    # NB: s_quad/c_quad must be default-arg-bound: the
                    # generator scope is shared across heads, so free-var
                    # lookup at emit time would find the last head's defs
                    def first_squad(es_prev=es_prev, s_quad=s_quad):
                        es_prev[0] = s_quad(0)
                    if p > 0:
                        yield first_squad, True
                        for qd in range(1, p):
                            def step(qd=qd, es_prev=es_prev, s_quad=s_quad,
                                     c_quad=c_quad):
                                es_cur = s_quad(qd)
                                c_quad(qd - 1, es_prev[0])
                                es_prev[0] = es_cur
                            yield step, True

                        def last_cq(es_prev=es_prev, p=p, c_quad=c_quad):
                            c_quad(p - 1, es_prev[0])
                        yield last_cq, True

                    def diag_ctx(tl=tl, h=h, p=p):
                        cp = tl["cp"]
                        for strip in range(4):
                            for c in range(strip + 1):
                                jc = 4 * p + c
                                if strip < 2:
                                    es_sl = tl["esA"][:, strip,
                                                      128 * c:128 * c + 128]
                                else:
                                    es_sl = tl["esB"][:, strip - 2,
                                                      128 * c:128 * c + 128]
                                nc.tensor.matmul(
                                    cp[:, 128 * strip:128 * strip + 128],
                                    vext[:, jc, h, 0:65], es_sl,
                                    start=(p == 0 and c == 0),
                                    stop=(c == strip),
                                    skip_group_check=True)
                    yield diag_ctx, True

                    def normalize(tl=tl, pb=pb, hc=hc, i0=i0,
                                  halves=(p == NST - 1 and h == HLOC - 1)):
                        cp = tl["cp"]
                        n = 2 if halves else 1
                        w = ST // n
                        for x in range(n):
                            xs = slice(w * x, w * x + w)
                            rs = spool.tile([1, ST], F32, tag="rs", bufs=2,
                                            name="rs")
                            nc.vector.tensor_copy(out=rs[0:1, 0:w],
                                                  in_=cp[64:65, xs])
                            rc = spool.tile([1, ST], F32, tag="rc", bufs=2,
                                            name="rc")
                            nc.vector.reciprocal_approx_fast(
                                out=rc[0:1, 0:w], in_=rs[0:1, 0:w])
                            rb = spool.tile([64, ST], F32, tag="rb", bufs=2,
                                            name="rb")
                            nc.gpsimd.partition_broadcast(rb[0:64, 0:w],
                                                          rc[0:1, 0:w])
                            nc.vector.tensor_tensor(
                                out=ctxT[pb:pb + 64, hc,
                                         i0 + w * x:i0 + w * x + w],
                                in0=cp[0:64, xs], in1=rb[0:64, 0:w],
                                op=mybir.AluOpType.mult)
                    yield normalize, True

            def run_phase(p, other_units, pre_units=None):
                """Emit attn(p), inserting other_units evenly at
                injectable points."""
                steps = list(attn_steps_causal(p))
                if p == 0:
                    # heads are [prep, diag_ctx, normalize] triples: run
                    # preps two heads deep (cp/es pools are 2-3 deep) with
                    # tile-0's v-proj units before the first diag-ctx
                    # (whose vext reads need all four chunk writes emitted)
                    for emit, _ in (steps[0], steps[3]):
                        emit()
                    for u in (pre_units or []):
                        u()
                    steps = [steps[i] for i in
                             (1, 2, 6, 4, 5, 9, 7, 8, 10, 11)]
                ninj = sum(1 for _, inj in steps if inj)
                credit = 0.0
                # +4 virtual points: leave a few units to emit after the
                # final normalize, covering its latency before the next
                # phase / tail output projection
                per = len(other_units) / max(1, ninj + (8 if p == NST - 1 else 4))
                oi = 0
                for emit, inj in steps:
                    emit()
                    if inj:
                        credit += per
                        while oi < len(other_units) and credit >= 1.0:
                            other_units[oi]()
                            oi += 1
                            credit -= 1.0
                while oi < len(other_units):
                    other_units[oi]()
                    oi += 1

            if variant == "causal":
                xs = (xq_t0, xk_t0, xv_t0)
                units0 = proj_units(0, xs)
                # only q/k-proj gate attention(0): emit them as the head,
                # and let tile-0's v-proj stream/compute during phase 0
                for u in units0[0:4]:
                    u()
                nc.scalar.dma_start(wo_sb[:], wo[:])
                # filler assignment: outproj(0) into p1, outproj(1)+(2)
                # into p3 (which has no proj work and is exp-bound)
                for p in range(NST):
                    other = []
                    if p + 1 < NST:
                        xs = emit_x_loads(p + 1)
                        other += proj_units(p + 1, xs)
                    if p == 1:
                        other += outproj_units(0)
                    elif p == 3:
                        other += outproj_units(1, act_drain=False)
                        other += outproj_units(2, act_drain=False)
                    run_phase(p, other, pre_units=units0[4:8] if p == 0
                              else None)
                for u in outproj_units(NST - 1, last=True):
                    u()
            else:
                # full attention reads all of kT/vext: all projections
                # first, then attn/outproj staggered (original schedule)
                for t in range(NST):
                    if t > 0:
                        xs = emit_x_loads(t)
                    else:
                        xs = (xq_t0, xk_t0, xv_t0)
                    for u in proj_units(t, xs):
                        u()
                nc.scalar.dma_start(wo_sb[:], wo[:])

                def attn_full(it):
                    i0 = ST * it
                    for h in range(HLOC):
                        pb = 64 * (h % 2)
                        hc = h // 2
                        cp = ctxpool.tile([65, ST], F32, tag="ctx")
                        qrep = epool.tile([128, ST], BF16, tag="qrep", bufs=3)
                        nc.vector.tensor_copy(
                            out=qrep[0:64, :],
                            in_=qT[pb:pb + 64, hc, i0:i0 + ST])
                        nc.vector.tensor_copy(
                            out=qrep[64:128, :],
                            in_=qT[pb:pb + 64, hc, i0:i0 + ST])

                        def s_quad(qd, h=h, qrep=qrep, i0=i0):
                            tiles = []
                            for half in range(2):
                                jc0 = 4 * qd + 2 * half
                                sp = app.tile([128, 2, ST], F32, tag="mm")
                                for c in range(2):
                                    jc = jc0 + c
                                    nc.tensor.matmul(
                                        sp[:, c, :],
                                        kT[:, h, 128 * jc:128 * jc + 128],
                                        qrep[:], start=True, stop=True)
                                if variant == "general":
                                    mt = mpool.tile([128, 2, ST], F32,
                                                    tag="mask", bufs=2)
                                    nc.sync.dma_start(
                                        mt[:],
                                        maskTn.rearrange(
                                            "(a b) i -> b a i", b=128)
                                        [:, jc0:jc0 + 2, i0:i0 + ST])
                                    nc.vector.tensor_tensor(
                                        out=sp[:], in0=sp[:], in1=mt[:],
                                        op=mybir.AluOpType.add)
                                es = epool.tile([128, 2, ST], BF16, tag="es")
                                nc.scalar.activation(es[:], sp[:], Exp)
                                tiles.append(es)
                            return tiles

                        def c_quad(qd, tiles, cp=cp, h=h):
                            for half, es in enumerate(tiles):
                                for c in range(2):
                                    jc = 4 * qd + 2 * half + c
                                    nc.tensor.matmul(
                                        cp[:], vext[:, jc, h, 0:65],
                                        es[:, c, :],
                                        start=(jc == 0), stop=(jc == JC - 1))

                        es_prev = s_quad(0)
                        for qd in range(1, JC // 4):
                            es_cur = s_quad(qd)
                            c_quad(qd - 1, es_prev)
                            es_prev = es_cur
                        c_quad(JC // 4 - 1, es_prev)
                        rs = spool.tile([1, ST], F32, tag="rs", bufs=2)
                        nc.vector.tensor_copy(out=rs[:], in_=cp[64:65, :])
                        rc = spool.tile([1, ST], F32, tag="rc", bufs=2)
                        nc.vector.reciprocal_approx_fast(out=rc[:], in_=rs[:])
                        rb = spool.tile([64, ST], F32, tag="rb", bufs=2)
                        nc.gpsimd.partition_broadcast(rb[:], rc[:])
                        nc.vector.tensor_tensor(
                            out=ctxT[pb:pb + 64, hc, i0:i0 + ST],
                            in0=cp[0:64, :], in1=rb[:],
                            op=mybir.AluOpType.mult)

                attn_full(0)
                for t in range(1, NST):
                    attn_full(t)
                    for u in outproj_units(t - 1):
                        u()
                for u in outproj_units(NST - 1, last=True):
                    u()

    nc.compile()
    return nc


def _get_nc(variant: str):
    if variant not in _NC_CACHE:
        _NC_CACHE[variant] = _build(variant)
    return _NC_CACHE[variant]


def kernel(**inputs) -> np.ndarray:
    global LAST_EXEC_TIME_NS
    q = np.asarray(inputs["query"], np.float32)
    k = np.asarray(inputs["key"], np.float32)
    v = np.asarray(inputs["value"], np.float32)
    mask = np.asarray(inputs["mask"], np.float32).reshape(S, S)
    Wq = np.asarray(inputs["Wq"], np.float32)
    bq = np.asarray(inputs["bq"], np.float32)
    Wk = np.asarray(inputs["Wk"], np.float32)
    bk = np.asarray(inputs["bk"], np.float32)
    Wv = np.asarray(inputs["Wv"], np.float32)
    bv = np.asarray(inputs["bv"], np.float32)
    Wo = np.asarray(inputs["Wo"], np.float32)
    bo = np.asarray(inputs["bo"], np.float32)

    if not mask.any():
        variant = "zeros"
    elif np.array_equal(mask, np.triu(np.ones((S, S), np.float32), k=1)):
        variant = "causal"
    else:
        variant = "general"

    scale = np.float32(1.0 / np.sqrt(DH) / 2.0)  # /2: replicated-K scores double
    xT = {}
    for b in range(B):
        xT[("q", b)] = _xT_layout(q[b])
        xT[("k", b)] = _xT_layout(k[b])
        xT[("v", b)] = _xT_layout(v[b])
    if variant == "general":
        maskTn_np = np.ascontiguousarray(mask.T) * np.float32(-1e9)

    in_maps = []
    for c in range(NCORES):
        b, g = divmod(c, 4)
        sl = slice(g * DLOC, (g + 1) * DLOC)
        m = {
            "xqT": xT[("q", b)],
            "xkT": xT[("k", b)],
            "xvT": xT[("v", b)],
            "wq": _w_layout(Wq[:, sl] * scale),
            "wk": _w_layout(Wk[:, sl]),
            "wv": _w_layout(Wv[:, sl]),
            "wo": _w_layout(Wo[sl, :], out=True),
            "bq2": np.ascontiguousarray((bq[sl] * scale).reshape(2, 128).T),
            "bk2": np.ascontiguousarray(bk[sl].reshape(2, 128).T),
            "bv1": bv[sl].reshape(1, DLOC),
        }
        if variant == "general":
            m["maskTn"] = maskTn_np
        in_maps.append(m)

    nc = _get_nc(variant)
    trace = bool(os.environ.get("BASS_TRACE"))
    res = run_bass_kernel_spmd(nc, in_maps, core_ids=list(range(NCORES)),
                               trace=trace)
    LAST_EXEC_TIME_NS = res.exec_time_ns

    out = np.empty((B, S, D), np.float32)
    for b in range(B):
        acc = np.zeros((S, D), np.float64)
        for g in range(4):
            acc += np.asarray(res.results[4 * b + g]["out"], np.float64)
        out[b] = (acc + bo).astype(np.float32)
    return out


# revision 27
# speedup vs baseline: 1.0175x; 1.0175x over previous
"""Multi-head attention (B=2, S=2048, D=1024, H=16, causal-mask capable)
on 8 Trainium2 NeuronCores.

Sharding: batch x head-group tensor parallel. Core c handles batch b=c//4
and head group g=c%4 (4 heads, d' slice of 256). Wq/Wk/Wv are split
column-wise per head group, Wo row-wise; per-core partial outputs are
summed on host (plus bo).

Device dataflow (per core), all matmul operands in bf16 (1 cyc/row on the
PE at any moving size; rounding error ~0.2% per operand, far inside the
2e-2 budget; all accumulation stays fp32 in PSUM):
  - host supplies x^T (=[D, S]) per batch so contraction dims land on
    SBUF partitions with no on-device transposes; bf16 halves DMA bytes
  - qT/kT [d', s] and v [s, d'] projections accumulate over D in PSUM;
    q/v biases added on DVE, the 8 replicated k-bias writes on the ACT
    engine (Identity + per-partition bias AP) which idles during proj
  - scores^T[j, i] = kT^T-slice @ qT-slice per 128-key chunk; ACT exp
    (no max-subtraction needed: |scores| <~ 8 for unit-variance data)
  - causal diagonal blocks are sub-tiled into 128-query strips (drops
    ~15% of the score/ctx matmul rows) and masked by one static
    lower-triangle bf16 multiply on DVE instead of gpsimd affine_select
  - ctx^T accumulates v-chunk^T @ expS with an appended ones column so
    row 64 of PSUM carries the softmax denominator; normalize: DVE copy
    out of PSUM + fast reciprocal (NOT from PSUM: the custom DVE op
    misreads PSUM on hw) + gpsimd partition_broadcast + DVE multiply
  - output projection ctx^T-chunks @ Wo-chunks; bf16 partial [S, D] to
    HBM, upcast + summed on host
The causal schedule interleaves projection/output-projection work units
between attention score/ctx quads so the in-order PE queue always has
work that does not depend on the latest exp; PSUM is budgeted to exactly
8 banks (attn pairs 2x2 + proj/outproj 2x1 + ctx 2x1). DMA triggers cost
~600-800ns of sequencer time each and are spread across the Sync/Scalar/
GpSimd sequencers; constants are DVE-memset-built, and a dummy
activation at t=0 pre-loads the ACT function table (exp/identity/copy
share one table) during the DMA head.
"""

import os
import sys

import numpy as np

try:
    import concourse.bass as bass  # noqa: F401
except ImportError:
    sys.path.insert(0, "/opt/trn_rl_repo")

import concourse.bass as bass
import concourse.tile as tile
from concourse import bacc, mybir
from concourse.bass_utils import run_bass_kernel_spmd

# Optional NTFF profiling hook (only used when BASS_TRACE=1): the agent
# image's antenv package lacks axon_hooks, so register an equivalent.
try:
    import antenv.axon_hooks  # noqa: F401
except ImportError:
    try:
        import types

        import trn_agent_boot.trn_boot as _tb

        _h = _tb._ntff_profile_via_ctypes("/opt/axon/libaxon_pjrt.so")
        _m = types.ModuleType("antenv.axon_hooks")
        _m.get_axon_ntff_profile_hook = lambda: _h
        _m.set_axon_ntff_profile_hook = lambda h: None
        sys.modules["antenv.axon_hooks"] = _m
    except Exception:
        pass

B, S, D, H = 2, 2048, 1024, 16
DH = 64                 # head dim
HLOC = 4                # heads per core
DLOC = HLOC * DH        # 256 d' per core
KC = 8                  # contraction chunks of 128 over D
ST = 512                # s-tile (matmul moving size)
NST = S // ST           # 4
JC = S // 128           # 16 key chunks
NCORES = 8

F32 = mybir.dt.float32
BF16 = mybir.dt.bfloat16
NPBF16 = mybir.dt.np(BF16)

LAST_EXEC_TIME_NS = None
_NC_CACHE = {}


def _to_bf16(x: np.ndarray) -> np.ndarray:
    """Round fp32 -> bf16 nearest-even, returned as ml_dtypes.bfloat16."""
    u = np.ascontiguousarray(x, np.float32).view(np.uint32)
    lsb = (u >> np.uint32(16)) & np.uint32(1)
    out = ((u + np.uint32(0x7FFF) + lsb) >> np.uint32(16)).astype(np.uint16)
    return out.view(NPBF16)


def _xT_layout(x2d: np.ndarray) -> np.ndarray:
    """[S, D] -> [128, NST, KC, ST] with X[p,t,kc,s] = x[t*ST+s, kc*128+p],
    bf16. Gives 8KB-contiguous per-partition DMA descriptors."""
    v = x2d.reshape(NST, ST, KC, 128).transpose(3, 0, 2, 1)
    return _to_bf16(np.ascontiguousarray(v))


def _w_layout(w: np.ndarray, out: bool = False) -> np.ndarray:
    """[D, DLOC] -> [128, KC, DLOC] (or [DLOC, D] -> [128, 2, D]) with
    partition = contraction index within its 128-chunk; bf16."""
    kc = 2 if out else KC
    v = w.reshape(kc, 128, w.shape[1]).transpose(1, 0, 2)
    return _to_bf16(np.ascontiguousarray(v))


def _build(variant: str):
    """variant: 'causal' | 'zeros' | 'general'"""
    nc = bacc.Bacc("TRN2", target_bir_lowering=False, debug=False)

    xqT = nc.declare_dram_parameter("xqT", [128, NST, KC, ST], BF16, isOutput=False)
    xkT = nc.declare_dram_parameter("xkT", [128, NST, KC, ST], BF16, isOutput=False)
    xvT = nc.declare_dram_parameter("xvT", [128, NST, KC, ST], BF16, isOutput=False)
    wq = nc.declare_dram_parameter("wq", [128, KC, DLOC], BF16, isOutput=False)
    wk = nc.declare_dram_parameter("wk", [128, KC, DLOC], BF16, isOutput=False)
    wv = nc.declare_dram_parameter("wv", [128, KC, DLOC], BF16, isOutput=False)
    wo = nc.declare_dram_parameter("wo", [128, 2, D], BF16, isOutput=False)
    bq2 = nc.declare_dram_parameter("bq2", [128, 2], F32, isOutput=False)
    bk2 = nc.declare_dram_parameter("bk2", [128, 2], F32, isOutput=False)
    bv1 = nc.declare_dram_parameter("bv1", [1, DLOC], F32, isOutput=False)
    if variant == "general":
        maskTn = nc.declare_dram_parameter("maskTn", [S, S], F32, isOutput=False)
    out_d = nc.declare_dram_parameter("out", [S, D], BF16, isOutput=True)

    Exp = mybir.ActivationFunctionType.Exp
    Ident = mybir.ActivationFunctionType.Identity

    with tile.TileContext(nc) as tc:
        with tc.tile_pool(name="wpool", bufs=1) as wpool, \
             tc.tile_pool(name="xpool", bufs=1) as xpool, \
             tc.tile_pool(name="epool", bufs=3) as epool, \
             tc.tile_pool(name="opool", bufs=2) as opool, \
             tc.tile_pool(name="spool", bufs=1) as spool, \
             tc.tile_pool(name="mpool", bufs=1) as mpool, \
             tc.tile_pool(name="app", bufs=2, space="PSUM") as app, \
             tc.tile_pool(name="ppp", bufs=2, space="PSUM") as ppp, \
             tc.tile_pool(name="ctxp", bufs=2, space="PSUM") as ctxpool:

            # ---- startup: memset-built constants, ACT table prewarm,
            # DMA wavefront spread over sequencers, PE warmup ----
            vext = wpool.tile([128, JC, HLOC, 65], BF16, tag="vext")
            nc.vector.memset(vext[:, :, :, 64], 1.0)
            warm_sb = wpool.tile([128, 32], BF16, tag="warm")
            nc.vector.memset(warm_sb[:], 1.0)
            # Static masks for the sub-tiled diagonal block-row. Strip o
            # (128 queries) of a 512-query i-tile attends chunks c<=o of the
            # diagonal block-row; block (o,c) is full for c<o, in-block
            # lower-triangle for c==o, zero for c>o. One affine iota
            # 128*(strip-c)+(s-p) >= 0 encodes all three cases. mA covers
            # strips 0-1 (c in 0..1), mB strips 2-3 (c in 0..3).
            mA = wpool.tile([128, 2, 256], BF16, tag="mA")
            nc.vector.memset(mA[:], 1.0)
            nc.gpsimd.affine_select(
                out=mA[:].rearrange("p o (c s) -> p o c s", s=128),
                in_=mA[:].rearrange("p o (c s) -> p o c s", s=128),
                pattern=[[128, 2], [-128, 2], [1, 128]],
                compare_op=mybir.AluOpType.is_ge, fill=0.0,
                base=0, channel_multiplier=-1)
            mB = wpool.tile([128, 2, 512], BF16, tag="mB")
            nc.vector.memset(mB[:], 1.0)
            nc.gpsimd.affine_select(
                out=mB[:].rearrange("p o (c s) -> p o c s", s=128),
                in_=mB[:].rearrange("p o (c s) -> p o c s", s=128),
                pattern=[[128, 2], [-128, 4], [1, 128]],
                compare_op=mybir.AluOpType.is_ge, fill=0.0,
                base=256, channel_multiplier=-1)
            actwarm = wpool.tile([1, 8], F32, tag="actwarm")
            nc.scalar.activation(actwarm[:], warm_sb[0:1, 0:8], Exp)

            wq_sb = wpool.tile([128, KC, DLOC], BF16, tag="wq")
            wk_sb = wpool.tile([128, KC, DLOC], BF16, tag="wk")
            wv_sb = wpool.tile([128, KC, DLOC], BF16, tag="wv")
            wo_sb = wpool.tile([128, 2, D], BF16, tag="wo")
            bq_sb = wpool.tile([128, 2], F32, tag="bq")
            bk_sb = wpool.tile([128, 2], F32, tag="bk")
            bv_sb = wpool.tile([1, DLOC], F32, tag="bv")
            bvb = wpool.tile([128, DLOC], F32, tag="bvb")
            xq_t0 = xpool.tile([128, KC, ST], BF16, tag="xq")
            xk_t0 = xpool.tile([128, KC, ST], BF16, tag="xk")
            xv_t0 = xpool.tile([128, KC, ST], BF16, tag="xv")
            # first q-proj matmul needs only wq chunk 0 + xq chunk 0
            for kc2 in range(KC // 2):
                nc.scalar.dma_start(
                    wq_sb[:, 2 * kc2:2 * kc2 + 2, :],
                    wq[:, 2 * kc2:2 * kc2 + 2, :])
            for kc in range(KC):
                nc.sync.dma_start(
                    xq_t0[:, kc:kc + 1, :], xqT[:, 0, kc:kc + 1, :])
            nc.scalar.dma_start(bq_sb[:], bq2[:])
            nc.scalar.dma_start(bk_sb[:], bk2[:])
            nc.scalar.dma_start(wk_sb[:], wk[:])
            for half in range(2):
                nc.gpsimd.dma_start(
                    xk_t0[:, 4 * half:4 * half + 4, :],
                    xkT[:, 0, 4 * half:4 * half + 4, :])
            nc.scalar.dma_start(bv_sb[:], bv1[:])
            nc.scalar.dma_start(wv_sb[:], wv[:])
            for half in range(2):
                nc.gpsimd.dma_start(
                    xv_t0[:, 4 * half:4 * half + 4, :],
                    xvT[:, 0, 4 * half:4 * half + 4, :])
            nc.gpsimd.partition_broadcast(bvb[:], bv_sb[:])

            warm_ps = ppp.tile([128, ST], F32, tag="pj")
            # enough to bridge engine-boot (~7us) to first-chunk arrival
            # (~6us) and ramp the PE p-state; more would block the in-order
            # PE queue past data arrival
            for i in range(40):
                nc.tensor.matmul(
                    warm_ps[0:32, 0:32], warm_sb[:], warm_sb[:],
                    start=True, stop=True, skip_group_check=True)

            # persistent activation tensors
            qT = wpool.tile([128, 2, S], BF16, tag="qT")
            # kT stored replicated per head ([0:64] == [64:128]) so scores
            # matmuls run at K=128 like everything else (K-switches ~430ns)
            kT = wpool.tile([128, HLOC, S], BF16, tag="kT")
            ctxT = wpool.tile([128, 2, S], BF16, tag="ctxT")

            def emit_x_loads(t):
                xq_t = xpool.tile([128, KC, ST], BF16, tag="xq")
                xk_t = xpool.tile([128, KC, ST], BF16, tag="xk")
                xv_t = xpool.tile([128, KC, ST], BF16, tag="xv")
                nc.sync.dma_start(xq_t[:], xqT[:, t])
                nc.sync.dma_start(xk_t[:], xkT[:, t])
                nc.sync.dma_start(xv_t[:], xvT[:, t])
                return xq_t, xk_t, xv_t

            def proj_units(t, xs):
                """8 PE work units of ~0.9-1.8us each for tile t."""
                s0 = ST * t
                xq_t, xk_t, xv_t = xs

                def qk_unit(dst, dc, w_sb, b_sb, x_t, k_on_act=True):
                    def emit():
                        ps = ppp.tile([128, ST], F32, tag="pj")
                        for kc in range(KC):
                            nc.tensor.matmul(
                                ps[:], w_sb[:, kc, 128 * dc:128 * dc + 128],
                                x_t[:, kc, :],
                                start=(kc == 0), stop=(kc == KC - 1))
                        if dst is qT:
                            nc.vector.tensor_scalar_add(
                                out=dst[:, dc, s0:s0 + ST], in0=ps[:],
                                scalar1=b_sb[:, dc:dc + 1])
                        else:
                            # replicated k-bias writes (gpsimd cannot read
                            # PSUM): ACT when its phase is exp-light,
                            # else DVE
                            for half in range(2):
                                hsl = slice(64 * half, 64 * half + 64)
                                for rep in range(2):
                                    dsl = dst[64 * rep:64 * rep + 64,
                                              2 * dc + half, s0:s0 + ST]
                                    if k_on_act:
                                        nc.scalar.activation(
                                            dsl, ps[hsl, :], Ident,
                                            bias=b_sb[hsl, dc:dc + 1])
                                    else:
                                        nc.vector.tensor_scalar_add(
                                            out=dsl, in0=ps[hsl, :],
                                            scalar1=b_sb[hsl, dc:dc + 1])
                    return emit

                def v_unit(sc):
                    def emit():
                        ps = ppp.tile([128, ST], F32, tag="pj")
                        for kc in range(KC):
                            nc.tensor.matmul(
                                ps[:, 0:DLOC],
                                xv_t[:, kc, 128 * sc:128 * sc + 128],
                                wv_sb[:, kc, :],
                                start=(kc == 0), stop=(kc == KC - 1))
                        jc = 4 * t + sc
                        nc.vector.tensor_tensor(
                            out=vext[:, jc, :, 0:64],
                            in0=ps[:, 0:DLOC].rearrange(
                                "p (h d) -> p h d", d=DH),
                            in1=bvb[:].rearrange("p (h d) -> p h d", d=DH),
                            op=mybir.AluOpType.add)
                    return emit

                units = []
                units.append(qk_unit(qT, 0, wq_sb, bq_sb, xq_t))
                units.append(qk_unit(kT, 0, wk_sb, bk_sb, xk_t))
                units.append(qk_unit(qT, 1, wq_sb, bq_sb, xq_t))
                units.append(qk_unit(kT, 1, wk_sb, bk_sb, xk_t))
                for sc in range(4):
                    units.append(v_unit(sc))
                return units

            def outproj_units(it, last=False, act_drain=True):
                """4 sc groups x 2 et units of 2 matmuls + drain + store."""
                i0 = ST * it
                units = []
                obs = [None] * 4

                def op_unit(sc, et):
                    def emit():
                        if obs[sc] is None:
                            obs[sc] = opool.tile([128, D], BF16, tag="ob",
                                                 name="ob")
                        ob = obs[sc]
                        ps = ppp.tile([128, ST], F32, tag="pj")
                        for dc in range(2):
                            nc.tensor.matmul(
                                ps[:],
                                ctxT[:, dc, i0 + 128 * sc:i0 + 128 * sc + 128],
                                wo_sb[:, dc, ST * et:ST * et + ST],
                                start=(dc == 0), stop=(dc == 1))
                        if et == 0 and act_drain:
                            nc.scalar.copy(ob[:, ST * et:ST * et + ST], ps[:])
                        else:
                            nc.vector.tensor_copy(
                                out=ob[:, ST * et:ST * et + ST], in_=ps[:])
                        if et == 1:
                            nc.sync.dma_start(
                                out_d[i0 + 128 * sc:i0 + 128 * sc + 128, :],
                                ob[:])
                    return emit

                for sc in range(4):
                    for et in range(2):
                        units.append(op_unit(sc, et))
                return units

            def attn_steps_causal(p):
                """Generator of (emit_fn, injectable) steps for i-tile p.
                injectable=True marks points where foreign PE work may be
                inserted without hurting the score->exp->ctx pipeline."""
                i0 = ST * p
                for h in range(HLOC):
                    pb = 64 * (h % 2)
                    hc = h // 2
                    # tiles are allocated lazily at emission time (pool
                    # buffer-rotation deps only see already-emitted users)
                    tl = {}

                    def prep(h=h, pb=pb, hc=hc, tl=tl, i0=i0, p=p):
                        tl["cp"] = ctxpool.tile([65, ST], F32, tag="ctx",
                                                name="cp")
                        tl["qrep"] = epool.tile([128, ST], BF16, tag="qrep",
                                                bufs=3, name="qrep")
                        tl["esA"] = epool.tile([128, 2, 256], BF16, tag="esA",
                                               bufs=2, name="esA")
                        tl["esB"] = epool.tile([128, 2, 512], BF16, tag="esB",
                                               bufs=2, name="esB")
                        qrep, esA, esB = tl["qrep"], tl["esA"], tl["esB"]
                        nc.vector.tensor_copy(
                            out=qrep[0:64, :], in_=qT[pb:pb + 64, hc, i0:i0 + ST])
                        nc.vector.tensor_copy(
                            out=qrep[64:128, :], in_=qT[pb:pb + 64, hc, i0:i0 + ST])
                        # diagonal block-row, sub-tiled into 128-query
                        # strips: strip o needs chunks c<=o; computed as
                        # strips 0-1 x chunks 0-1 (sdA) and strips 2-3 x
                        # chunks 0-3 (sdB), masked by the static mA/mB
                        sdA = ppp.tile([128, ST], F32, tag="pj", name="sdA")
                        for o in range(2):
                            for c in range(2):
                                jc = 4 * p + c
                                nc.tensor.matmul(
                                    sdA[:, 256 * o + 128 * c:
                                        256 * o + 128 * c + 128],
                                    kT[:, h, 128 * jc:128 * jc + 128],
                                    qrep[:, 128 * o:128 * o + 128],
                                    start=True, stop=True,
                                    skip_group_check=True)
                        sdB = app.tile([128, 2, ST], F32, tag="mm")
                        for o in range(2):
                            for c in range(4):
                                jc = 4 * p + c
                                nc.tensor.matmul(
                                    sdB[:, o, 128 * c:128 * c + 128],
                                    kT[:, h, 128 * jc:128 * jc + 128],
                                    qrep[:, 256 + 128 * o:256 + 128 * o + 128],
                                    start=True, stop=True,
                                    skip_group_check=True)
                        nc.scalar.activation(
                            esA[:], sdA[:].rearrange("p (o x) -> p o x", o=2),
                            Exp)
                        nc.vector.tensor_tensor(
                            out=esA[:], in0=esA[:], in1=mA[:],
                            op=mybir.AluOpType.mult)
                        nc.scalar.activation(esB[:], sdB[:], Exp)
                        nc.vector.tensor_tensor(
                            out=esB[:], in0=esB[:], in1=mB[:],
                            op=mybir.AluOpType.mult)
                    yield prep, True

                    def s_quad(qd, h=h, tl=tl):
                        tiles = []
                        for half in range(2):
                            jc0 = 4 * qd + 2 * half
                            sp = app.tile([128, 2, ST], F32, tag="mm",
                                          name="sp")
                            for c in range(2):
                                jc = jc0 + c
                                nc.tensor.matmul(
                                    sp[:, c, :],
                                    kT[:, h, 128 * jc:128 * jc + 128],
                                    tl["qrep"][:],
                                    start=True, stop=True,
                                    skip_group_check=True)
                            es = epool.tile([128, 2, ST], BF16, tag="es",
                                            name="es")
                            nc.scalar.activation(es[:], sp[:], Exp)
                            tiles.append(es)
                        return tiles

                    def c_quad(qd, tiles, tl=tl, h=h):
                        for half, es in enumerate(tiles):
                            for c in range(2):
                                jc = 4 * qd + 2 * half + c
                                nc.tensor.matmul(
                                    tl["cp"][:], vext[:, jc, h, 0:65],
                                    es[:, c, :],
                                    start=(jc == 0), stop=False,
                                    skip_group_check=True)

                    es_prev = [None]

                    # NB: s_quad/c_quad must be default-arg-bound: the
                    # generator scope is shared across heads, so free-var
                    # lookup at emit time would find the last head's defs
                    def first_squad(es_prev=es_prev, s_quad=s_quad):
                        es_prev[0] = s_quad(0)
                    if p > 0:
                        yield first_squad, True
                        for qd in range(1, p):
                            def step(qd=qd, es_prev=es_prev, s_quad=s_quad,
                                     c_quad=c_quad):
                                es_cur = s_quad(qd)
                                c_quad(qd - 1, es_prev[0])
                                es_prev[0] = es_cur
                            yield step, True

                        def last_cq(es_prev=es_prev, p=p, c_quad=c_quad):
                            c_quad(p - 1, es_prev[0])
                        yield last_cq, True

                    def diag_ctx(tl=tl, h=h, p=p):
                        cp = tl["cp"]
                        for strip in range(4):
                            for c in range(strip + 1):
                                jc = 4 * p + c
                                if strip < 2:
                                    es_sl = tl["esA"][:, strip,
                                                      128 * c:128 * c + 128]
                                else:
                                    es_sl = tl["esB"][:, strip - 2,
                                                      128 * c:128 * c + 128]
                                nc.tensor.matmul(
                                    cp[:, 128 * strip:128 * strip + 128],
                                    vext[:, jc, h, 0:65], es_sl,
                                    start=(p == 0 and c == 0),
                                    stop=(c == strip),
                                    skip_group_check=True)
                    yield diag_ctx, True

                    def normalize(tl=tl, pb=pb, hc=hc, i0=i0,
                                  halves=(p == NST - 1 and h == HLOC - 1)):
                        cp = tl["cp"]
                        n = 2 if halves else 1
                        w = ST // n
                        for x in range(n):
                            xs = slice(w * x, w * x + w)
                            rs = spool.tile([1, ST], F32, tag="rs", bufs=2,
                                            name="rs")
                            nc.vector.tensor_copy(out=rs[0:1, 0:w],
                                                  in_=cp[64:65, xs])
                            rc = spool.tile([1, ST], F32, tag="rc", bufs=2,
                                            name="rc")
                            nc.vector.reciprocal_approx_fast(
                                out=rc[0:1, 0:w], in_=rs[0:1, 0:w])
                            rb = spool.tile([64, ST], F32, tag="rb", bufs=2,
                                            name="rb")
                            nc.gpsimd.partition_broadcast(rb[0:64, 0:w],
                                                          rc[0:1, 0:w])
                            nc.vector.tensor_tensor(
                                out=ctxT[pb:pb + 64, hc,
                                         i0 + w * x:i0 + w * x + w],
                                in0=cp[0:64, xs], in1=rb[0:64, 0:w],
                                op=mybir.AluOpType.mult)
                    yield normalize, True

            def run_phase(p, other_units, pre_units=None):
                """Emit attn(p), inserting other_units evenly at
                injectable points."""
                steps = list(attn_steps_causal(p))
                if p == 0:
                    # heads are [prep, diag_ctx, normalize] triples: run
                    # preps two heads deep (cp/es pools are 2-3 deep) with
                    # tile-0's v-proj units before the first diag-ctx
                    # (whose vext reads need all four chunk writes emitted)
                    for emit, _ in (steps[0], steps[3]):
                        emit()
                    for u in (pre_units or []):
                        u()
                    steps = [steps[i] for i in
                             (1, 2, 6, 4, 5, 9, 7, 8, 10, 11)]
                ninj = sum(1 for _, inj in steps if inj)
                credit = 0.0
                # +4 virtual points: leave a few units to emit after the
                # final normalize, covering its latency before the next
                # phase / tail output projection
                per = len(other_units) / max(1, ninj + (8 if p == NST - 1 else 4))
                oi = 0
                for emit, inj in steps:
                    emit()
                    if inj:
                        credit += per
                        while oi < len(other_units) and credit >= 1.0:
                            other_units[oi]()
                            oi += 1
                            credit -= 1.0
                while oi < len(other_units):
                    other_units[oi]()
                    oi += 1

            if variant == "causal":
                xs = (xq_t0, xk_t0, xv_t0)
                units0 = proj_units(0, xs)
                # only q/k-proj gate attention(0): emit them as the head,
                # and let tile-0's v-proj stream/compute during phase 0
                for u in units0[0:4]:
                    u()
                nc.scalar.dma_start(wo_sb[:], wo[:])
                # filler assignment: outproj(0) into p1, outproj(1)+(2)
                # into p3 (which has no proj work and is exp-bound)
                for p in range(NST):
                    other = []
                    if p + 1 < NST:
                        xs = emit_x_loads(p + 1)
                        other += proj_units(p + 1, xs)
                    if p == 1:
                        other += outproj_units(0)
                    elif p == 3:
                        other += outproj_units(1, act_drain=False)
                        other += outproj_units(2, act_drain=False)
                    run_phase(p, other, pre_units=units0[4:8] if p == 0
                              else None)
                for u in outproj_units(NST - 1, last=True):
                    u()
            else:
                # full attention reads all of kT/vext: all projections
                # first, then attn/outproj staggered (original schedule)
                for t in range(NST):
                    if t > 0:
                        xs = emit_x_loads(t)
                    else:
                        xs = (xq_t0, xk_t0, xv_t0)
                    for u in proj_units(t, xs):
                        u()
                nc.scalar.dma_start(wo_sb[:], wo[:])

                def attn_full(it):
                    i0 = ST * it
                    for h in range(HLOC):
                        pb = 64 * (h % 2)
                        hc = h // 2
                        cp = ctxpool.tile([65, ST], F32, tag="ctx")
                        qrep = epool.tile([128, ST], BF16, tag="qrep", bufs=3)
                        nc.vector.tensor_copy(
                            out=qrep[0:64, :],
                            in_=qT[pb:pb + 64, hc, i0:i0 + ST])
                        nc.vector.tensor_copy(
                            out=qrep[64:128, :],
                            in_=qT[pb:pb + 64, hc, i0:i0 + ST])

                        def s_quad(qd, h=h, qrep=qrep, i0=i0):
                            tiles = []
                            for half in range(2):
                                jc0 = 4 * qd + 2 * half
                                sp = app.tile([128, 2, ST], F32, tag="mm")
                                for c in range(2):
                                    jc = jc0 + c
                                    nc.tensor.matmul(
                                        sp[:, c, :],
                                        kT[:, h, 128 * jc:128 * jc + 128],
                                        qrep[:], start=True, stop=True)
                                if variant == "general":
                                    mt = mpool.tile([128, 2, ST], F32,
                                                    tag="mask", bufs=2)
                                    nc.sync.dma_start(
                                        mt[:],
                                        maskTn.rearrange(
                                            "(a b) i -> b a i", b=128)
                                        [:, jc0:jc0 + 2, i0:i0 + ST])
                                    nc.vector.tensor_tensor(
                                        out=sp[:], in0=sp[:], in1=mt[:],
                                        op=mybir.AluOpType.add)
                                es = epool.tile([128, 2, ST], BF16, tag="es")
                                nc.scalar.activation(es[:], sp[:], Exp)
                                tiles.append(es)
                            return tiles

                        def c_quad(qd, tiles, cp=cp, h=h):
                            for half, es in enumerate(tiles):
                                for c in range(2):
                                    jc = 4 * qd + 2 * half + c
                                    nc.tensor.matmul(
                                        cp[:], vext[:, jc, h, 0:65],
                                        es[:, c, :],
                                        start=(jc == 0), stop=(jc == JC - 1))

                        es_prev = s_quad(0)
                        for qd in range(1, JC // 4):
                            es_cur = s_quad(qd)
                            c_quad(qd - 1, es_prev)
                            es_prev = es_cur
                        c_quad(JC // 4 - 1, es_prev)
                        rs = spool.tile([1, ST], F32, tag="rs", bufs=2)
                        nc.vector.tensor_copy(out=rs[:], in_=cp[64:65, :])
                        rc = spool.tile([1, ST], F32, tag="rc", bufs=2)
                        nc.vector.reciprocal_approx_fast(out=rc[:], in_=rs[:])
                        rb = spool.tile([64, ST], F32, tag="rb", bufs=2)
                        nc.gpsimd.partition_broadcast(rb[:], rc[:])
                        nc.vector.tensor_tensor(
                            out=ctxT[pb:pb + 64, hc, i0:i0 + ST],
                            in0=cp[0:64, :], in1=rb[:],
                            op=mybir.AluOpType.mult)

                attn_full(0)
                for t in range(1, NST):
                    attn_full(t)
                    for u in outproj_units(t - 1):
                        u()
                for u in outproj_units(NST - 1, last=True):
                    u()

    nc.compile()
    return nc


def _get_nc(variant: str):
    if variant not in _NC_CACHE:
        _NC_CACHE[variant] = _build(variant)
    return _NC_CACHE[variant]


def kernel(**inputs) -> np.ndarray:
    global LAST_EXEC_TIME_NS
    q = np.asarray(inputs["query"], np.float32)
    k = np.asarray(inputs["key"], np.float32)
    v = np.asarray(inputs["value"], np.float32)
    mask = np.asarray(inputs["mask"], np.float32).reshape(S, S)
    Wq = np.asarray(inputs["Wq"], np.float32)
    bq = np.asarray(inputs["bq"], np.float32)
    Wk = np.asarray(inputs["Wk"], np.float32)
    bk = np.asarray(inputs["bk"], np.float32)
    Wv = np.asarray(inputs["Wv"], np.float32)
    bv = np.asarray(inputs["bv"], np.float32)
    Wo = np.asarray(inputs["Wo"], np.float32)
    bo = np.asarray(inputs["bo"], np.float32)

    if not mask.any():
        variant = "zeros"
    elif np.array_equal(mask, np.triu(np.ones((S, S), np.float32), k=1)):
        variant = "causal"
    else:
        variant = "general"

    scale = np.float32(1.0 / np.sqrt(DH) / 2.0)  # /2: replicated-K scores double
    xT = {}
    for b in range(B):
        xT[("q", b)] = _xT_layout(q[b])
        xT[("k", b)] = _xT_layout(k[b])
        xT[("v", b)] = _xT_layout(v[b])
    if variant == "general":
        maskTn_np = np.ascontiguousarray(mask.T) * np.float32(-1e9)

    in_maps = []
    for c in range(NCORES):
        b, g = divmod(c, 4)
        sl = slice(g * DLOC, (g + 1) * DLOC)
        m = {
            "xqT": xT[("q", b)],
            "xkT": xT[("k", b)],
            "xvT": xT[("v", b)],
            "wq": _w_layout(Wq[:, sl] * scale),
            "wk": _w_layout(Wk[:, sl]),
            "wv": _w_layout(Wv[:, sl]),
            "wo": _w_layout(Wo[sl, :], out=True),
            "bq2": np.ascontiguousarray((bq[sl] * scale).reshape(2, 128).T),
            "bk2": np.ascontiguousarray(bk[sl].reshape(2, 128).T),
            "bv1": bv[sl].reshape(1, DLOC),
        }
        if variant == "general":
            m["maskTn"] = maskTn_np
        in_maps.append(m)

    nc = _get_nc(variant)
    trace = bool(os.environ.get("BASS_TRACE"))
    res = run_bass_kernel_spmd(nc, in_maps, core_ids=list(range(NCORES)),
                               trace=trace)
    LAST_EXEC_TIME_NS = res.exec_time_ns

    out = np.empty((B, S, D), np.float32)
    for b in range(B):
        acc = np.zeros((S, D), np.float64)
        for g in range(4):
            acc += np.asarray(res.results[4 * b + g]["out"], np.float64)
        out[b] = (acc + bo).astype(np.float32)
    return out
